# revision 57
# baseline (speedup 1.0000x reference)
"""Multi-head attention kernel for Trainium2, 8-core tensor/data parallel.

Problem: x[2,2048,1024] -> qkv proj (w_qkv [1024,3072]) -> 16-head attention
         -> out proj (w_proj [1024,1024]) + b_proj.

Sharding: core c handles batch b=c//4 and heads 4*(c%4)..4*(c%4)+4.
Each core computes a partial output Y^T = w_proj_rows^T @ OH (its 4 heads'
contribution, transposed); the host sums the 4 partials per batch,
transposes, and adds the bias.

Design (ACT/PE co-bound software pipeline, measured 225us vs 362us for the
serial-stage baseline):
- All inputs bf16 (halves input DMA, enables FWL weight loads).
- Attention runs as 8 blocks (head-pair ht x 512-query block).  Per key-tile
  the pipeline is S-matmuls (PE) -> exp (ACT) -> PV-matmuls (PE), emitted so
  the PE queue holds the next S before the previous PV: ACT stays saturated
  while the PE shadows it.  The two heads' S matmuls use disjoint PE row
  groups (rows 0:64 / 64:128) — the array runs them concurrently (~2-3x the
  serialized rate for 64-deep contractions).
- The HAM clock gate is kept open: a junk-matmul warm-up during the input
  DMA wait un-throttles the PE before the projection chase, and stage-1/3
  work is spread through the blocks as filler bursts so the PE never idles
  past the ~3.4us re-throttle window.
- Stage-1: K projections accumulate ct-outer chasing the x DMA chunks; Q/V
  projections and the output projection ride the attention blocks as PE
  fillers; only the last query block's output projection trails the final
  norm.
- Softmax row sums come free via a ones-column appended to V.  Per-head
  normalization: reciprocal_approx_fast on the [1,512] rowsum row (DVE, ~5x
  faster than the iterative reciprocal; needs an SBUF input), GPSIMD
  partition_broadcast (attn ucode library) for the per-query reciprocal
  broadcast (no PE/PSUM involvement), one copy+multiply into the bf16 O^T
  staging tile.
- fp8 (e4m3, DoubleRow) PV was measured at ~2.9e-2 rel err — the attention
  output shrinks with key-averaging as fast as the quantization noise, so
  fp8 P/V does not average down.  bf16 it is.
- PSUM budget: tag "st" 2x[128,2,512]f32 (4 banks; stage-1 groups, S tiles,
  fillers, stage-3) + tag "ot" 4x[65,512] (4 banks; per-head PV
  accumulators, double-buffered across blocks) = 8 banks.
"""

from contextlib import ExitStack

import numpy as np

import concourse.bass as bass
import concourse.mybir as mybir
from concourse import bacc, library_config, tile

B, N, C, H = 2, 2048, 1024, 16
D = C // H            # 64 head dim
SCALE = float(D) ** -0.5
HPC = 4               # heads per core
HD = HPC * D          # 256 head-dim columns per core
NCORES = 8

F32 = mybir.dt.float32
F32R = mybir.dt.float32r
BF16 = mybir.dt.bfloat16

USE_FAST_RECIP = True
# fp8e4m3 DoubleRow PV measures ~2.9e-2 rel err (P and V quantization each
# contribute ~2e-2; attention output shrinks with key-averaging as fast as
# the quantization noise, so fp8 does not average down) — keep bf16.
PV_FP8 = False
OUT_BF16 = True       # bf16 partial-output DMA (halves the tail drain)

FP8 = mybir.dt.float8e4

CT = C // 128         # 8 channel 128-tiles
KT = N // 128         # 16 key 128-tiles
QW = 1024             # query block width
QB = N // QW          # 2 query blocks
HT = 2                # head-pair tiles (2 heads of 64 dims each)


def _build():
    nc = bacc.Bacc(None)
    xT = nc.declare_dram_parameter("xT", [C, N], BF16, isOutput=False)
    wq = nc.declare_dram_parameter("wq", [C, HD], BF16, isOutput=False)
    wk = nc.declare_dram_parameter("wk", [C, HD], BF16, isOutput=False)
    wv = nc.declare_dram_parameter("wv", [C, HD], BF16, isOutput=False)
    wp = nc.declare_dram_parameter("wp", [HD, C], BF16, isOutput=False)
    yT = nc.declare_dram_parameter("yT", [C, N], BF16 if OUT_BF16 else F32,
                                   isOutput=True)

    with tile.TileContext(nc) as tc, ExitStack() as ctx:
        const_pool = ctx.enter_context(tc.tile_pool(name="const", bufs=1))
        w_pool = ctx.enter_context(tc.tile_pool(name="w", bufs=1))
        x_pool = ctx.enter_context(tc.tile_pool(name="x", bufs=1))
        qk_pool = ctx.enter_context(tc.tile_pool(name="qk", bufs=1))
        vo_pool = ctx.enter_context(tc.tile_pool(name="vo", bufs=1))
        oht_pool = ctx.enter_context(tc.tile_pool(name="oht", bufs=1))
        pt_pool = ctx.enter_context(tc.tile_pool(name="pt", bufs=4))
        rs_pool = ctx.enter_context(tc.tile_pool(name="rs", bufs=2))
        out_pool = ctx.enter_context(tc.tile_pool(name="out", bufs=4))
        psA = ctx.enter_context(tc.tile_pool(name="psA", bufs=2, space="PSUM"))
        psB = ctx.enter_context(tc.tile_pool(name="psB", bufs=2, space="PSUM"))

        # gpsimd "attn" library provides partition_broadcast for the
        # softmax-denominator broadcast (keeps the norm chain off PE/PSUM)
        nc.gpsimd.load_library(library_config.attn)

        exp_bias = const_pool.tile([128, 1], F32, name="exp_bias")
        nc.vector.memset(exp_bias, -2.0 if PV_FP8 else 0.0)

        # PE warm-up: ~20 junk matmuls during the input-DMA wait so the HAM
        # clock gate opens (~3.4us of sustained activity) before the
        # projection chase starts — the whole prefix then runs at 2.4 GHz.
        warm_src = const_pool.tile([128, 512], BF16, name="warm_src")
        nc.vector.memset(warm_src, 0.0)
        warm_ps = psA.tile([128, 512], F32, name="warm_ps", tag="st")
        for _ in range(20):
            nc.tensor.matmul(warm_ps, warm_src[:, 0:128], warm_src)

        # ---- input DMAs (weights for K first, then x chunks, then rest) ----
        wk_t = w_pool.tile([128, CT, HD], BF16, name="wk", tag="wk")
        nc.sync.dma_start(
            out=wk_t, in_=wk[:, :].rearrange("(ct p) h -> p ct h", p=128))
        wq_t = w_pool.tile([128, CT, HD], BF16, name="wq", tag="wq")
        nc.sync.dma_start(
            out=wq_t, in_=wq[:, :].rearrange("(ct p) h -> p ct h", p=128))
        # x streams in query-major pieces so the prefix projection groups
        # complete per query-chunk instead of after the whole load
        x_t = x_pool.tile([128, CT, N], BF16, name="xT", tag="xT")
        for qc in range(4):
            qs = slice(qc * 512, (qc + 1) * 512)
            for ct in range(CT):
                nc.sync.dma_start(
                    out=x_t[:, ct, qs], in_=xT[ct * 128:(ct + 1) * 128, qs])
        wv_t = w_pool.tile([128, CT, HD], BF16, name="wv", tag="wv")
        nc.sync.dma_start(
            out=wv_t, in_=wv[:, :].rearrange("(ct p) h -> p ct h", p=128))
        wp_t = w_pool.tile([128, HT, C], BF16, name="wp", tag="wp")
        nc.sync.dma_start(
            out=wp_t, in_=wp[:, :].rearrange("(ht p) c -> p ht c", p=128))

        # ---- persistent activations ----
        qT = [qk_pool.tile([128, N], BF16, name=f"qT{i}", tag=f"qT{i}")
              for i in range(HT)]
        kT = [qk_pool.tile([128, N], BF16, name=f"kT{i}", tag=f"kT{i}")
              for i in range(HT)]
        # V with a ones column appended per head: [128 keys, 4 heads, 64+1]
        vo = [vo_pool.tile([128, HPC, D + 1], BF16, name=f"vo{i}",
                           tag=f"vo{i}") for i in range(KT)]
        oht = [oht_pool.tile([128, N], BF16, name=f"oht{i}", tag=f"oht{i}")
               for i in range(HT)]
        for t in vo:
            nc.vector.memset(t, 1.0)

        # ---- stage-1 emitters (each is one PSUM group on the "st" ring) ----
        def emit_qk_group(dst, w_t, ht, qb):
            ps = psA.tile([128, 2, 512], F32, name="proj", tag="st")
            for ct in range(CT):
                for j in range(2):
                    js = slice(qb * QW + j * 512, qb * QW + (j + 1) * 512)
                    nc.tensor.matmul(
                        ps[:, j, :], w_t[:, ct, ht * 128:(ht + 1) * 128],
                        x_t[:, ct, js], start=(ct == 0), stop=(ct == CT - 1))
            qs = slice(qb * QW, (qb + 1) * QW)
            nc.vector.tensor_copy(
                dst[ht][:, qs], ps.rearrange("p j q -> p (j q)"))

        def emit_v_group(kt):
            ks = slice(kt * 128, (kt + 1) * 128)
            ps = psA.tile([128, HD], F32, name="vproj", tag="st")
            for ct in range(CT):
                nc.tensor.matmul(ps, x_t[:, ct, ks], wv_t[:, ct, :],
                                 start=(ct == 0), stop=(ct == CT - 1))
            nc.vector.tensor_copy(
                vo[kt][:, :, 0:D], ps.rearrange("p (h d) -> p h d", h=HPC))

        # ---- attention block: head pair ht, 512-query block qv ----
        # Both heads of the pair run per key-tile with row-disjoint S
        # matmuls (hp0 rows 0:64, hp1 rows 64:128) — the PE overlaps
        # row-disjoint matmuls, ~2-3x the serialized rate.
        def emit_block(ht, qv, fillers):
            qs = slice(qv * 512, (qv + 1) * 512)
            ot = [psB.tile([D + 1, 512], F32, name=f"ot{hp}", tag="ot",
                           bufs=4)
                  for hp in range(2)]
            pts = {}

            def emit_pv(kt):
                for hp in range(2):
                    nc.tensor.matmul(
                        ot[hp], vo[kt][:, 2 * ht + hp, :],
                        pts[kt][:, hp * 512:(hp + 1) * 512],
                        start=(kt == 0), stop=(kt == KT - 1))
                del pts[kt]

            for kt in range(KT):
                st = psA.tile([128, 2, 512], F32, name="st", tag="st")
                for hp in range(2):
                    prow = slice(hp * 64, hp * 64 + 64)
                    nc.tensor.matmul(
                        st[:, hp, :], kT[ht][prow, kt * 128:(kt + 1) * 128],
                        qT[ht][prow, qs])
                pts[kt] = pt_pool.tile([128, QW], BF16, name="pt", tag="pt")
                nc.scalar.activation(
                    pts[kt], st, mybir.ActivationFunctionType.Exp,
                    scale=SCALE, bias=exp_bias)
                # PE filler burst for this kt (stage-1/3 work riding the
                # exp shadow)
                fill = fillers.get(kt)
                if fill is not None:
                    fill()
                # PV of the previous kt (keeps ACT saturated: the PE queue
                # holds the next S before the PV, so the S runs during the
                # exp and the PV right after it)
                if kt > 0:
                    emit_pv(kt - 1)
            emit_pv(KT - 1)

            # normalization per head: O^T rows * (1/rowsum) broadcast
            for hp in range(2):
                prow = slice(hp * 64, hp * 64 + 64)
                rinv = rs_pool.tile([1, 512], F32, name="rinv", tag="rinv")
                if USE_FAST_RECIP:
                    rsum_sb = rs_pool.tile([1, 512], F32, name="rsum",
                                           tag="rsum")
                    nc.vector.tensor_copy(rsum_sb, ot[hp][D:D + 1, :])
                    nc.vector.reciprocal_approx_fast(rinv, rsum_sb)
                else:
                    with nc.allow_low_precision(reason="softmax denom"):
                        nc.vector.reciprocal(rinv, ot[hp][D:D + 1, :])
                rb = rs_pool.tile([128, 512], F32, name="rb", tag="rb")
                nc.gpsimd.partition_broadcast(rb, rinv)
                dst = oht[ht][prow, qs]
                nc.vector.tensor_copy(dst, ot[hp][0:D, :])
                nc.vector.tensor_mul(dst, dst, rb[prow, :])

        # ---- stage 3 emitter: one (ct, qb) output tile group ----
        def emit_s3_group(ct, qb):
            cs = slice(ct * 128, (ct + 1) * 128)
            qs = slice(qb * QW, (qb + 1) * QW)
            ps = psA.tile([128, 2, 512], F32, name="y", tag="st")
            for ht in range(HT):
                for j in range(2):
                    js = slice(qb * QW + j * 512, qb * QW + (j + 1) * 512)
                    nc.tensor.matmul(
                        ps[:, j, :], wp_t[:, ht, cs], oht[ht][:, js],
                        start=(ht == 0), stop=(ht == HT - 1))
            o = out_pool.tile([128, QW], BF16 if OUT_BF16 else F32,
                              name="yo", tag="yo")
            nc.vector.tensor_copy(o, ps.rearrange("p j q -> p (j q)"))
            nc.sync.dma_start(out=yT[cs, qs], in_=o)

        # Tail variant: j halves on separate psum rings (st + the freed ot
        # ring) for depth, copies alternating between ACT and DVE so neither
        # serializes the drain.
        def emit_s3_tail(ct, qb):
            cs = slice(ct * 128, (ct + 1) * 128)
            qs = slice(qb * QW, (qb + 1) * QW)
            ps_j = [psA.tile([128, 512], F32, name="yj0", tag="st"),
                    psB.tile([128, 512], F32, name="yj1", tag="ot", bufs=4)]
            for j in range(2):
                js = slice(qb * QW + j * 512, qb * QW + (j + 1) * 512)
                for ht in range(HT):
                    nc.tensor.matmul(
                        ps_j[j], wp_t[:, ht, cs], oht[ht][:, js],
                        start=(ht == 0), stop=(ht == HT - 1))
            o = out_pool.tile([128, QW], BF16 if OUT_BF16 else F32,
                              name="yo", tag="yo")
            for j in range(2):
                dst = o[:, j * 512:(j + 1) * 512]
                if (ct + j) % 2 == 0:
                    nc.scalar.copy(dst, ps_j[j])
                else:
                    nc.vector.tensor_copy(dst, ps_j[j])
            nc.sync.dma_start(out=yT[cs, qs], in_=o)

        # ---- emission schedule ----
        # qb-outer block order; stage-1 groups and stage-3(qb0) ride the
        # blocks as PE filler bursts so the PE never idles long (HAM-warm)
        # and ACT starts as early as possible.
        # Prefix: both K(ht0) groups (st psum ring) and Q(ht0, qb0) as two
        # j-halves (borrowing idle ot-ring slots) accumulate ct-outer so all
        # three finish one matmul after the last x chunk lands (chasing the
        # input DMA).
        psK = [psA.tile([128, 2, 512], F32, name=f"psK{g}", tag="st")
               for g in range(2)]
        psQ = [psB.tile([128, 512], F32, name=f"psQ{j}", tag="ot", bufs=4)
               for j in range(2)]
        psK1 = [psB.tile([128, 512], F32, name=f"psK1{j}", tag="ot", bufs=4)
                for j in range(2)]
        for qc in range(4):
            qs = slice(qc * 512, (qc + 1) * 512)
            for ct in range(CT):
                nc.tensor.matmul(
                    psK[qc // 2][:, qc % 2, :], wk_t[:, ct, 0:128],
                    x_t[:, ct, qs], start=(ct == 0), stop=(ct == CT - 1))
                if qc < 2:
                    nc.tensor.matmul(
                        psQ[qc], wq_t[:, ct, 0:128], x_t[:, ct, qs],
                        start=(ct == 0), stop=(ct == CT - 1))
                    nc.tensor.matmul(
                        psK1[qc], wk_t[:, ct, 128:256], x_t[:, ct, qs],
                        start=(ct == 0), stop=(ct == CT - 1))
            # cast each half as soon as its chunk's accumulation finishes
            nc.vector.tensor_copy(kT[0][:, qs], psK[qc // 2][:, qc % 2, :])
            if qc < 2:
                nc.vector.tensor_copy(qT[0][:, qs], psQ[qc])
                nc.vector.tensor_copy(kT[1][:, qs], psK1[qc])
        emit_v_group(0)
        emit_v_group(1)

        def v_fillers():
            return {kt: (lambda k=kt: emit_v_group(k + 2))
                    for kt in range(KT - 2)}

        # blocks are (ht head-pair, qv 512-query block)
        blocks = [(0, 0), (0, 1), (1, 0), (1, 1),
                  (0, 2), (1, 2), (0, 3), (1, 3)]
        fill_plan = {
            0: v_fillers(),
            1: {4: lambda: emit_qk_group(kT, wk_t, 1, 1),
                11: lambda: emit_qk_group(qT, wq_t, 1, 0)},
            2: {5: lambda: emit_qk_group(qT, wq_t, 0, 1)},
            3: {5: lambda: emit_qk_group(qT, wq_t, 1, 1)},
            4: {6: lambda: emit_s3_group(0, 0),
                12: lambda: emit_s3_group(1, 0)},
            5: {3: lambda: emit_s3_group(2, 0),
                11: lambda: emit_s3_group(3, 0)},
            6: {3: lambda: emit_s3_group(4, 0),
                8: lambda: emit_s3_group(5, 0),
                13: lambda: emit_s3_group(6, 0)},
        }
        for bi, (ht, qv) in enumerate(blocks):
            emit_block(ht, qv, fill_plan.get(bi, {}))

        # Warm-keepers: PE work that is runnable immediately after the last
        # block's PV (gated only by earlier norms) so the HAM doesn't
        # re-throttle during the final norm chains.
        emit_s3_group(7, 0)

        # ---- stage-3 tail: last query block ----
        for ct in range(CT):
            emit_s3_tail(ct, 1)

    nc.finalize()
    return nc


_NC_CACHE = None
TRACE = False
LAST_RESULTS = None


def _get_nc():
    global _NC_CACHE
    if _NC_CACHE is None:
        _NC_CACHE = _build()
    return _NC_CACHE


def kernel(x, w_qkv, w_proj, b_proj):
    global LAST_RESULTS
    import ml_dtypes
    from concourse.bass_utils import run_bass_kernel_spmd

    bf16 = ml_dtypes.bfloat16
    x = np.asarray(x, dtype=np.float32)
    w_qkv = np.asarray(w_qkv, dtype=np.float32)
    w_proj = np.asarray(w_proj, dtype=np.float32)
    b_proj = np.asarray(b_proj, dtype=np.float32)

    nc = _get_nc()
    xT_b = [np.ascontiguousarray(x[b].T).astype(bf16) for b in range(B)]
    in_maps = []
    for c in range(NCORES):
        b, g = divmod(c, NCORES // B)
        hs = slice(g * HD, (g + 1) * HD)
        in_maps.append({
            "xT": xT_b[b],
            "wq": np.ascontiguousarray(w_qkv[:, 0 * C:1 * C][:, hs]).astype(bf16),
            "wk": np.ascontiguousarray(w_qkv[:, 1 * C:2 * C][:, hs]).astype(bf16),
            "wv": np.ascontiguousarray(w_qkv[:, 2 * C:3 * C][:, hs]).astype(bf16),
            "wp": np.ascontiguousarray(w_proj[g * HD:(g + 1) * HD, :]).astype(bf16),
        })
    res = run_bass_kernel_spmd(nc, in_maps, list(range(NCORES)), trace=TRACE)
    LAST_RESULTS = res
    out = np.empty((B, N, C), dtype=np.float32)
    ncb = NCORES // B
    for b in range(B):
        acc = np.asarray(res.results[b * ncb]["yT"]).astype(np.float32)
        for g in range(1, ncb):
            acc += np.asarray(res.results[b * ncb + g]["yT"]).astype(np.float32)
        out[b] = acc.T + b_proj
    return out
